# revision 1
# baseline (speedup 1.0000x reference)
"""DetectionLoss Trainium2 Bass kernel.

Data-parallel over batch: 2 images per core x 8 cores; host sums 18 partial
sums per core (npos is a global normalizer, so per-core normalization is
impossible anyway - the sharding hint's "per-shard sums + counts").

Device algorithm per core:
  sparse path (starts immediately): box cells -> 128x128 same-cell masks
  (last-box-wins winners, min-label targets) -> indirect gathers of the
  per-cell records (obj, reg0..3) and cls logit at the target class ->
  smooth-L1 and CE numerators.
  dense path (overlapped): sum_k exp(cls[k, cell]) for every cell via
  bf16 matmul against a block-selector, staged to DRAM, gathered back at
  the <=128 positive cells for the logsumexp term; softplus over all obj
  logits via Exp+Ln(x+1) (gen3 ACT tables lack Softplus).

The obj+reg inputs are repacked on host into per-cell records [2HW, 5]
(pure relayout - all arithmetic happens on device) so one indirect DMA per
scale fetches all five values per box; indirect DMAs cost ~1.1us each on
GPSIMD and were the dominant serial chain in v1.
"""

import numpy as np
import ml_dtypes

import concourse.bass as bass
import concourse.tile as tile
from concourse import bacc, mybir
from concourse.bass_utils import run_bass_kernel_spmd
from concourse.tile_rust import add_dep_helper

F32 = mybir.dt.float32
BF16 = mybir.dt.bfloat16
I32 = mybir.dt.int32
AF = mybir.ActivationFunctionType
OP = mybir.AluOpType
AX = mybir.AxisListType

B_TOT = 16
N_CORES = 8
B_SH = B_TOT // N_CORES
NBOX = 64
NP = B_SH * NBOX  # 128 partitions: (image, box)
C = 30
SCALES = [(80, 80), (40, 40), (20, 20)]
BIG = 1.0e9
CHUNK = 400  # divides every HW/2; psum [4*nch, 400] fits one bank

CLS_W, REG_W, OBJ_W = 1.0, 5.0, 1.0
NPART = 18  # per scale s, cols 6s + [lse, clsval, sl1, obj, softplus, npos]


def _consts():
    ident = np.eye(128, dtype=np.float32)
    utri = np.triu(np.ones((128, 128), np.float32), 1)
    big = np.concatenate([ident, utri], axis=1)  # [128, 256]

    p = np.arange(128)
    bvec = (p >= NBOX).astype(np.float32)
    kc = np.zeros((128, 24), np.float32)
    for s, (h, w) in enumerate(SCALES):
        hw = h * w
        kc[:, 0 + s] = w          # W
        kc[:, 3 + s] = h          # H
        kc[:, 6 + s] = w - 1
        kc[:, 9 + s] = h - 1
        kc[:, 12 + s] = bvec * hw          # key offset
        kc[:, 15 + s] = bvec * C * hw      # cls gather offset
        kc[:, 18 + s] = hw                 # for minlab*HW

    # [120, 4]: partition (b, k, u) -> column (b*2 + u)
    bsel = np.zeros((120, 4), ml_dtypes.bfloat16)
    for pp in range(120):
        b = pp // 60
        u = pp % 2
        bsel[pp, b * 2 + u] = 1.0

    ones = np.ones((128, 1), np.float32)
    return big, kc, bsel, ones


def emit(tc: tile.TileContext, outs, ins):
    """outs: partials AP [18]; ins: dict name -> AP (per-core shard shapes)."""
    nc = tc.nc
    out_ap = outs

    big_c, kc_c, bsel_c, ones_c = _consts()
    big_h = nc.inline_tensor(big_c, name="cbig")
    kc_h = nc.inline_tensor(kc_c, name="ckc")
    bsel_h = nc.inline_tensor(bsel_c, name="cbsel")
    ones_h = nc.inline_tensor(ones_c, name="cones")

    pools = []

    def mkpool(**kw):
        p = tc.alloc_tile_pool(**kw)
        pools.append(p)
        return p

    pool = mkpool(name="sb", bufs=1)
    seps = mkpool(name="seps", bufs=3, space="PSUM")
    kmps = mkpool(name="kmps", bufs=2, space="PSUM")
    lbps = mkpool(name="lbps", bufs=1, space="PSUM")
    fips = mkpool(name="fips", bufs=1, space="PSUM")

    # ---- tiny inputs first: the sparse chain is the critical path ----
    btile = pool.tile([NP, 4], F32, tag="btile")
    nc.sync.dma_start(out=btile[:], in_=ins["boxes"].rearrange("b n c -> (b n) c"))
    kct = pool.tile([128, 24], F32, tag="kct")
    nc.sync.dma_start(out=kct[:], in_=kc_h.ap())
    labi = pool.tile([NP, 1], I32, tag="labi")
    nc.sync.dma_start(out=labi[:], in_=ins["labels"].rearrange("b n -> (b n)")[:, None])
    bigt = pool.tile([128, 256], F32, tag="bigt")
    nc.sync.dma_start(out=bigt[:], in_=big_h.ap())
    utri = bigt[:, 128:256]
    bselt = pool.tile([120, 4], BF16, tag="bselt")
    nc.sync.dma_start(out=bselt[:], in_=bsel_h.ap())

    # ---- batched (all scales) box -> cell/key indices ----
    # floor(x) = round-to-nearest(x - 0.5): HW f32->i32 convert rounds.
    # gxy [128, (coord, scale)] does x and y for all 3 scales per op.
    kxy = kct[:, 0:6].rearrange("p (c s) -> p c s", c=2)
    kxy_clip = kct[:, 6:12].rearrange("p (c s) -> p c s", c=2)
    gr = pool.tile([NP, 2, 3], F32, tag="gr")
    nc.vector.tensor_tensor(
        out=gr[:], in0=btile[:, 0:2, None].to_broadcast([NP, 2, 3]), in1=kxy, op=OP.mult
    )
    nc.vector.tensor_scalar(out=gr[:], in0=gr[:], scalar1=-0.5, scalar2=None, op0=OP.add)
    gi = pool.tile([NP, 2, 3], I32, tag="gi")
    nc.vector.tensor_copy(out=gi[:], in_=gr[:])
    gf = pool.tile([NP, 2, 3], F32, tag="gf")
    nc.vector.tensor_copy(out=gf[:], in_=gi[:])
    nc.vector.tensor_tensor(out=gf[:], in0=gf[:], in1=kxy_clip, op=OP.min)

    cellf = pool.tile([NP, 3], F32, tag="cellf")
    nc.vector.tensor_tensor(out=cellf[:], in0=gf[:, 1, :], in1=kct[:, 0:3], op=OP.mult)
    nc.vector.tensor_add(cellf[:], cellf[:], gf[:, 0, :])
    keyf = pool.tile([NP, 3], F32, tag="keyf")
    nc.vector.tensor_add(keyf[:], cellf[:], kct[:, 12:15])
    keyi = pool.tile([NP, 3], I32, tag="keyi")
    nc.vector.tensor_copy(out=keyi[:], in_=keyf[:])

    # ---- obj+reg record gathers: issue as soon as keys exist ----
    og_all = pool.tile([NP, 15], F32, tag="og_all")  # (obj, reg0..3) x 3 scales
    for s in range(3):
        nc.gpsimd.indirect_dma_start(
            out=og_all[:, 5 * s : 5 * s + 5],
            out_offset=None,
            in_=ins[f"objreg{s}"],
            in_offset=bass.IndirectOffsetOnAxis(ap=keyi[:, s : s + 1], axis=0),
        )

    labf = pool.tile([NP, 1], F32, tag="labf")
    nc.vector.tensor_copy(out=labf[:], in_=labi[:])
    stack = pool.tile([128, NPART], F32, tag="stack")
    nc.vector.memset(stack[:], 0.0)
    stv = stack[:].rearrange("p (s j) -> p s j", j=6)

    # ---- key/label row matrices: PE transpose of broadcast columns ----
    # (labmat[p, q] = labf[q]; keymat_s[p, q] = keyf[q, s])
    labmat = lbps.tile([128, 128], F32, tag="labmat")
    nc.tensor.transpose(
        out=labmat[:], in_=labf[:].to_broadcast([128, 128]), identity=bigt[:, 0:128]
    )

    # ---- per-scale masks: winners (last box wins) + min same-cell label ----
    win3 = pool.tile([NP, 3], F32, tag="win3")
    minlab3 = pool.tile([NP, 3], F32, tag="minlab3")
    for s in range(3):
        kmat = kmps.tile([128, 128], F32, tag="kmat")
        nc.tensor.transpose(
            out=kmat[:],
            in_=keyf[:, s : s + 1].to_broadcast([128, 128]),
            identity=bigt[:, 0:128],
        )
        eqm = pool.tile([128, 128], F32, tag=f"eqm{s}")
        nc.vector.tensor_scalar(
            out=eqm[:], in0=kmat[:], scalar1=keyf[:, s : s + 1], scalar2=None, op0=OP.is_equal
        )
        lose = pool.tile([128, 128], F32, tag=f"lose{s}")
        nc.vector.tensor_mul(lose[:], eqm[:], utri)
        losev = pool.tile([NP, 1], F32, tag=f"losev{s}")
        nc.vector.tensor_reduce(out=losev[:], in_=lose[:], axis=AX.X, op=OP.max)
        nc.vector.tensor_scalar(
            out=win3[:, s : s + 1], in0=losev[:], scalar1=-1.0, scalar2=1.0, op0=OP.mult, op1=OP.add
        )
        cnd = pool.tile([128, 128], F32, tag=f"cnd{s}")
        nc.vector.tensor_scalar(
            out=cnd[:], in0=eqm[:], scalar1=-BIG, scalar2=BIG, op0=OP.mult, op1=OP.add
        )
        nc.vector.tensor_tensor(out=cnd[:], in0=cnd[:], in1=labmat[:], op=OP.add)
        nc.vector.tensor_reduce(out=minlab3[:, s : s + 1], in_=cnd[:], axis=AX.X, op=OP.min)

    cidxf = pool.tile([NP, 3], F32, tag="cidxf")
    nc.vector.tensor_tensor(out=cidxf[:], in0=minlab3[:], in1=kct[:, 18:21], op=OP.mult)
    nc.vector.tensor_add(cidxf[:], cidxf[:], cellf[:])
    nc.vector.tensor_add(cidxf[:], cidxf[:], kct[:, 15:18])
    cidxi = pool.tile([NP, 3], I32, tag="cidxi")
    nc.vector.tensor_copy(out=cidxi[:], in_=cidxf[:])

    # ---- cls-logit-at-target-class gathers ----
    clsv3 = pool.tile([NP, 3], F32, tag="clsv3")
    for s in range(3):
        nc.gpsimd.indirect_dma_start(
            out=clsv3[:, s : s + 1],
            out_offset=None,
            in_=ins[f"cls_p{s}"].rearrange("b k h w -> (b k h w)")[:, None],
            in_offset=bass.IndirectOffsetOnAxis(ap=cidxi[:, s : s + 1], axis=0),
        )

    # ---- dense phase, smallest scale first so its se-gather issues early.
    # cls loads go on the scalar HWDGE queue (sync queue holds the small
    # early loads + se writes); all Exp ACT ops are emitted before any Ln
    # to avoid ping-ponging activation-table loads (1.28us each).
    se_h = [
        nc.dram_tensor(f"se{s}", (B_SH * h * w,), F32, kind="Internal")
        for s, (h, w) in enumerate(SCALES)
    ]
    seg3 = pool.tile([NP, 3], F32, tag="seg3")
    obj_ln = []
    se_wr = {}
    for s, (H, W) in enumerate(SCALES):
        HW = H * W
        HW2 = HW // 2
        nch = HW2 // CHUNK if HW2 >= CHUNK else 1
        csz = HW2 // nch  # 400, 400, 200
        cls_pf = ins[f"cls_p{s}"].rearrange("b k (u f) w -> (b k u) (f w)", u=2)

        expt = pool.tile([120, HW2], BF16, tag=f"expt{s}")
        ndma = 2 if s == 0 else 1
        dsz = HW2 // ndma
        for di in range(ndma):
            ct = pool.tile([120, dsz], F32, tag=f"clsin{s}_{di}")
            nc.scalar.dma_start(out=ct[:], in_=cls_pf[:, di * dsz : (di + 1) * dsz])
            nc.scalar.activation(out=expt[:, di * dsz : (di + 1) * dsz], in_=ct[:], func=AF.Exp)

        # obj softplus: exp now, ln later (batched with the other Lns)
        p_obj = 128 if s < 2 else 32
        n_rec = B_SH * HW // p_obj
        objt = pool.tile([p_obj, n_rec * 5], F32, tag=f"objt{s}")
        nc.sync.dma_start(
            out=objt[:], in_=ins[f"objreg{s}"].rearrange("v r -> (v r)").rearrange("(p f) -> p f", p=p_obj)
        )
        objv = objt[:].rearrange("p (j r) -> p j r", r=5)[:, :, 0]
        obje = pool.tile([p_obj, n_rec], F32, tag=f"obje{s}")
        nc.scalar.activation(out=obje[:], in_=objv, func=AF.Exp)
        obj_ln.append((s, p_obj, n_rec, obje))

        sesb = pool.tile([4, HW2], F32, tag=f"sesb{s}")
        for ci in range(nch):
            se_ps = seps.tile([4, csz], F32, tag="seps")
            nc.tensor.matmul(
                out=se_ps[:],
                lhsT=bselt[:],
                rhs=expt[:, ci * csz : (ci + 1) * csz],
                start=True,
                stop=True,
            )
            nc.vector.tensor_copy(out=sesb[:, ci * csz : (ci + 1) * csz], in_=se_ps[:])
        # se flat layout is (b, u, j) = row-major [4, HW2]
        se_wr[s] = nc.sync.dma_start(
            out=se_h[s].ap().rearrange("(p f) -> p f", p=4), in_=sesb[:]
        )

    # se gathers ordered by expected write-completion time (s0's dense
    # pipeline is gated by the big cls0 transfer and finishes last)
    for s in (1, 2, 0):
        g = nc.gpsimd.indirect_dma_start(
            out=seg3[:, s : s + 1],
            out_offset=None,
            in_=se_h[s].ap()[:, None],
            in_offset=bass.IndirectOffsetOnAxis(ap=keyi[:, s : s + 1], axis=0),
        )
        add_dep_helper(g.ins, se_wr[s].ins, reason="se scratch RAW")

    # ---- smooth-L1 over gathered reg records (emitted late: depends on
    # gather DATA, which lands ~3us after issue under bulk-DMA contention;
    # anything DVE emitted after this would head-of-line stall) ----
    ogv = og_all[:].rearrange("p (s r) -> p s r", r=5)
    d12 = pool.tile([NP, 3, 4], F32, tag="d12")
    nc.vector.tensor_tensor(
        out=d12[:], in0=ogv[:, :, 1:5], in1=btile[:, None, :].to_broadcast([NP, 3, 4]), op=OP.subtract
    )
    nc.scalar.activation(out=d12[:], in_=d12[:], func=AF.Abs)
    q12 = pool.tile([NP, 3, 4], F32, tag="q12")
    nc.vector.tensor_scalar_min(q12[:], d12[:], 1.0)
    h12 = pool.tile([NP, 3, 4], F32, tag="h12")
    nc.vector.tensor_scalar(out=h12[:], in0=q12[:], scalar1=-0.5, scalar2=None, op0=OP.mult)
    nc.vector.tensor_add(h12[:], h12[:], d12[:])
    nc.vector.tensor_mul(h12[:], h12[:], q12[:])
    sl13 = pool.tile([NP, 3], F32, tag="sl13")
    nc.vector.tensor_reduce(out=sl13[:], in_=h12[:], axis=AX.X, op=OP.add)
    nc.vector.tensor_scalar(out=sl13[:], in0=sl13[:], scalar1=0.25, scalar2=None, op0=OP.mult)
    nc.vector.tensor_scalar_min(sl13[:], sl13[:], 10.0)
    nc.vector.tensor_mul(stv[:, :, 1], clsv3[:], win3[:])
    nc.vector.tensor_mul(stv[:, :, 2], sl13[:], win3[:])
    nc.vector.tensor_mul(stv[:, :, 3], ogv[:, :, 0], win3[:])
    nc.vector.tensor_copy(out=stv[:, :, 5], in_=win3[:])

    for s, p_obj, n_rec, obje in obj_ln:
        objl = pool.tile([p_obj, n_rec], F32, tag=f"objl{s}")
        nc.scalar.activation(
            out=objl[:], in_=obje[:], func=AF.Ln, bias=1.0,
            accum_out=stack[:p_obj, 6 * s + 4 : 6 * s + 5],
        )

    lse3 = pool.tile([NP, 3], F32, tag="lse3")
    nc.scalar.activation(out=lse3[:], in_=seg3[:], func=AF.Ln)
    nc.vector.tensor_mul(stv[:, :, 0], lse3[:], win3[:])

    # ---- final: transpose stack then sum along free (the v1 stack@ones
    # matmul showed a pathological 12us slice) ----
    finT = fips.tile([NPART, 128], F32, tag="finT")
    nc.tensor.transpose(out=finT[:], in_=stack[:], identity=bigt[:, 0:128])
    fin_sb = pool.tile([NPART, 1], F32, tag="fin_sb")
    nc.vector.tensor_reduce(out=fin_sb[:], in_=finT[:], axis=AX.X, op=OP.add)
    nc.sync.dma_start(out=out_ap, in_=fin_sb[:])

    for p in reversed(pools):
        p.release()


# ---------------------------------------------------------------------------
# host side
# ---------------------------------------------------------------------------

_CACHE = {}


def _build():
    if "nc" in _CACHE:
        return _CACHE["nc"]
    nc = bacc.Bacc(
        "TRN2",
        target_bir_lowering=False,
        debug=False,
        enable_asserts=False,
        num_devices=N_CORES,
    )
    ins = {}
    for s, (h, w) in enumerate(SCALES):
        ins[f"cls_p{s}"] = nc.dram_tensor(f"cls_p{s}", (B_SH, C, h, w), F32, kind="ExternalInput").ap()
        ins[f"objreg{s}"] = nc.dram_tensor(f"objreg{s}", (B_SH * h * w, 5), F32, kind="ExternalInput").ap()
    ins["boxes"] = nc.dram_tensor("boxes", (B_SH, NBOX, 4), F32, kind="ExternalInput").ap()
    ins["labels"] = nc.dram_tensor("labels", (B_SH, NBOX), I32, kind="ExternalInput").ap()
    out = nc.dram_tensor("partials", (NPART,), F32, kind="ExternalOutput").ap()

    with tile.TileContext(nc) as tc:
        emit(tc, out, ins)
    nc.compile()
    _CACHE["nc"] = nc
    return nc


def make_objreg(obj_slice, reg_slice):
    """[b,1,H,W] obj + [b,4,H,W] reg -> per-cell records [b*H*W, 5]."""
    b = obj_slice.shape[0]
    hw = obj_slice.shape[2] * obj_slice.shape[3]
    rec = np.empty((b * hw, 5), np.float32)
    rec[:, 0] = np.asarray(obj_slice).reshape(-1)
    rec[:, 1:] = np.asarray(reg_slice).reshape(b, 4, hw).transpose(0, 2, 1).reshape(b * hw, 4)
    return rec


def combine_partials(parts):
    """parts: [n_cores, 18] -> final [4] losses."""
    tot = np.asarray(parts, np.float64).sum(axis=0)
    cls_sum = reg_sum = obj_sum = 0.0
    for s, (h, w) in enumerate(SCALES):
        b = 6 * s
        lse, val, sl1, obj, sp, npos = tot[b : b + 6]
        npos = max(npos, 1.0)
        cls_sum += (lse - val) / npos * CLS_W
        reg_sum += sl1 / npos * REG_W
        obj_sum += (sp - obj) / (B_TOT * h * w) * OBJ_W
    cls_sum /= len(SCALES)
    reg_sum /= len(SCALES)
    obj_sum /= len(SCALES)
    total = cls_sum + reg_sum + obj_sum
    return np.array([total, cls_sum, reg_sum, obj_sum], np.float32)


TRACE = False
LAST_RESULT = None


def kernel(**inputs):
    global LAST_RESULT
    nc = _build()
    in_maps = []
    for c in range(N_CORES):
        lo, hi = c * B_SH, (c + 1) * B_SH
        m = {}
        for s in range(3):
            m[f"cls_p{s}"] = np.ascontiguousarray(inputs[f"cls_p{s}"][lo:hi])
            m[f"objreg{s}"] = make_objreg(
                inputs[f"obj_p{s}"][lo:hi], inputs[f"reg_p{s}"][lo:hi]
            )
        m["boxes"] = np.ascontiguousarray(inputs["boxes"][lo:hi])
        m["labels"] = np.ascontiguousarray(inputs["labels"][lo:hi])
        in_maps.append(m)
    res = run_bass_kernel_spmd(
        nc, in_maps, core_ids=list(range(N_CORES)), trace=TRACE
    )
    LAST_RESULT = res
    parts = np.stack([np.asarray(r["partials"]) for r in res.results])
    return combine_partials(parts)



# revision 8
# speedup vs baseline: 1.7640x; 1.7640x over previous
"""DetectionLoss Trainium2 Bass kernel (v2 - sparse-only).

Data-parallel over batch: 2 images per core x 8 cores; host sums 18 partial
sums per core (npos is a global normalizer, so per-core normalization is
impossible anyway - the sharding hint's "per-shard sums + counts").

v2 insight: the CE term only needs logsumexp at the <=128 positive cells,
so the whole dense sum-exp path of v1 (2.1MB cls DMA -> Exp -> 16 matmuls
-> DRAM staging -> gather) is replaced by host-packing per-cell records
[obj, reg0..3, cls0..29] and gathering 35-wide rows at the box cells.
lse is then exp+reduce+ln on a [128,3,30] tile. Dense work that remains:
softplus over all obj logits (67KB).

Other wins vs v1:
- one manual ACT table load of set 6 (natural_log_exp_and_others) serves
  every Exp and Ln: the compiler's auto-placement otherwise ping-pongs
  exp/ln tables at 1.28us per load (4 loads in v1).
- scalar_tensor_tensor fuses (key==key_q)*utri and (key!=key_q)*(BIG-lab)
  into single DVE ops.
- smooth-L1 and small copies run on gpsimd after its gathers; exp sums use
  activation accum_out; final partition-sum is a ones^T matmul.
"""

import numpy as np

import concourse.bass as bass
import concourse.tile as tile
from concourse import bacc, mybir

F32 = mybir.dt.float32
I32 = mybir.dt.int32
AF = mybir.ActivationFunctionType
OP = mybir.AluOpType
AX = mybir.AxisListType

B_TOT = 16
N_CORES = 8
B_SH = B_TOT // N_CORES
NBOX = 64
NP = B_SH * NBOX  # 128 partitions: (image, box)
C = 30
SCALES = [(80, 80), (40, 40), (20, 20)]
BIG = 1.0e9
REC_W = 5 + C  # obj, reg0..3, cls0..29
N_CELLS = sum(B_SH * h * w for h, w in SCALES)  # 16800
REC_BASE = [0, B_SH * 6400, B_SH * 6400 + B_SH * 1600]

CLS_W, REG_W, OBJ_W = 1.0, 5.0, 1.0
NPART = 18  # per scale s, cols 6s + [lse, clsval, sl1, obj, softplus, npos]

# act_info.json set 6 = natural_log_exp_and_others: serves Exp, Ln, Abs
ACT_SET_EXP_LN = 6


def _consts():
    ident = np.eye(128, dtype=np.float32)
    utri = np.triu(np.ones((128, 128), np.float32), 1)
    big = np.concatenate([ident, utri], axis=1)  # [128, 256]

    p = np.arange(128)
    bvec = (p >= NBOX).astype(np.float32)
    kc = np.zeros((128, 15 + C), np.float32)
    for s, (h, w) in enumerate(SCALES):
        hw = h * w
        kc[:, 0 + s] = w          # x multiplier
        kc[:, 3 + s] = h          # y multiplier
        kc[:, 6 + s] = w - 1      # x clip
        kc[:, 9 + s] = h - 1      # y clip
        kc[:, 12 + s] = bvec * hw + REC_BASE[s]  # record-row offset
    kc[:, 15:] = np.arange(C, dtype=np.float32)[None, :]  # iota over classes

    ones = np.ones((128, 1), np.float32)
    return big, kc, ones


def emit(tc: tile.TileContext, outs, ins):
    """outs: partials AP [18]; ins: dict name -> AP (per-core shard shapes)."""
    nc = tc.nc
    out_ap = outs

    big_c, kc_c, ones_c = _consts()
    big_h = nc.inline_tensor(big_c, name="cbig")
    kc_h = nc.inline_tensor(kc_c, name="ckc")
    ones_h = nc.inline_tensor(ones_c, name="cones")

    pools = []

    def mkpool(**kw):
        p = tc.alloc_tile_pool(**kw)
        pools.append(p)
        return p

    pool = mkpool(name="sb", bufs=1)
    tps = mkpool(name="tps", bufs=1, space="PSUM")
    fips = mkpool(name="fips", bufs=1, space="PSUM")

    # ---- single activation-table load serving all Exp AND Ln ops ----
    nc.scalar.add_instruction(mybir.InstLoadActFuncSet(
        name=nc.scalar.bass.get_next_instruction_name(),
        act_func_set_id=ACT_SET_EXP_LN,
        engine=mybir.EngineType.Activation, ins=[], outs=[]))

    # ---- input loads: sync queue carries the critical small tensors ----
    btile = pool.tile([NP, 4], F32, tag="btile")
    nc.sync.dma_start(out=btile[:], in_=ins["boxes"].rearrange("b n c -> (b n) c"))
    kct = pool.tile([128, 15 + C], F32, tag="kct")
    nc.sync.dma_start(out=kct[:], in_=kc_h.ap())
    labi = pool.tile([NP, 1], I32, tag="labi")
    nc.sync.dma_start(out=labi[:], in_=ins["labels"].rearrange("b n -> (b n)")[:, None])
    # gpsimd queue (idle until the gathers): transpose identity + ones
    bigt = pool.tile([128, 256], F32, tag="bigt")
    nc.gpsimd.dma_start(out=bigt[:], in_=big_h.ap())
    onest = pool.tile([128, 1], F32, tag="onest")
    nc.gpsimd.dma_start(out=onest[:], in_=ones_h.ap())
    utri = bigt[:, 128:256]
    # scalar queue: dense obj logits
    P2 = 32
    objt = []
    for s, (h, w) in enumerate(SCALES):
        n = B_SH * h * w
        p_obj = 128 if s < 2 else P2
        t = pool.tile([p_obj, n // p_obj], F32, tag=f"objt{s}")
        nc.scalar.dma_start(out=t[:], in_=ins[f"obj{s}"].rearrange("(p f) -> p f", p=p_obj))
        objt.append((p_obj, t))

    stack = pool.tile([128, NPART], F32, tag="stack")
    nc.vector.memset(stack[:], 0.0)
    stv = stack[:].rearrange("p (s j) -> p s j", j=6)

    # ---- box -> cell/key indices (vector) ----
    # floor(x) = round-to-nearest(x - 0.5): HW f32->i32 convert rounds.
    kxy = kct[:, 0:6].rearrange("p (c s) -> p c s", c=2)
    kxy_clip = kct[:, 6:12].rearrange("p (c s) -> p c s", c=2)
    gr = pool.tile([NP, 2, 3], F32, tag="gr")
    nc.vector.tensor_tensor(
        out=gr[:], in0=btile[:, 0:2, None].to_broadcast([NP, 2, 3]), in1=kxy, op=OP.mult
    )
    nc.vector.tensor_scalar(out=gr[:], in0=gr[:], scalar1=-0.5, scalar2=None, op0=OP.add)
    gi = pool.tile([NP, 2, 3], I32, tag="gi")
    nc.vector.tensor_copy(out=gi[:], in_=gr[:])
    gf = pool.tile([NP, 2, 3], F32, tag="gf")
    nc.vector.tensor_copy(out=gf[:], in_=gi[:])
    nc.vector.tensor_tensor(out=gf[:], in0=gf[:], in1=kxy_clip, op=OP.min)

    keyf = pool.tile([NP, 3], F32, tag="keyf")
    nc.vector.tensor_tensor(out=keyf[:], in0=gf[:, 1, :], in1=kct[:, 0:3], op=OP.mult)
    nc.vector.tensor_add(keyf[:], keyf[:], gf[:, 0, :])
    nc.vector.tensor_add(keyf[:], keyf[:], kct[:, 12:15])
    keyi = pool.tile([NP, 3], I32, tag="keyi")
    nc.vector.tensor_copy(out=keyi[:], in_=keyf[:])

    # ---- record gathers (gpsimd): one [128, 35] row per box per scale ----
    rows = pool.tile([NP, 3, REC_W], F32, tag="rows")
    for s in range(3):
        nc.gpsimd.indirect_dma_start(
            out=rows[:, s, :],
            out_offset=None,
            in_=ins["rec"],
            in_offset=bass.IndirectOffsetOnAxis(ap=keyi[:, s : s + 1], axis=0),
        )

    # ---- label helper columns + PE broadcast-transposes ----
    labf = pool.tile([NP, 1], F32, tag="labf")
    nc.vector.tensor_copy(out=labf[:], in_=labi[:])
    blf = pool.tile([NP, 1], F32, tag="blf")
    nc.vector.tensor_scalar(out=blf[:], in0=labf[:], scalar1=-1.0, scalar2=BIG, op0=OP.mult, op1=OP.add)

    # kmat_s[p, q] = keyf[q, s]; labmat[p, q] = labf[q]; blmat[p, q] = BIG - labf[q]
    kmat = []
    for s in range(3):
        km = tps.tile([128, 128], F32, tag=f"kmat{s}")
        nc.tensor.transpose(
            out=km[:], in_=keyf[:, s : s + 1].to_broadcast([128, 128]), identity=bigt[:, 0:128]
        )
        kmat.append(km)
    blmat = tps.tile([128, 128], F32, tag="blmat")
    nc.tensor.transpose(out=blmat[:], in_=blf[:].to_broadcast([128, 128]), identity=bigt[:, 0:128])
    labmat = tps.tile([128, 128], F32, tag="labmat")
    nc.tensor.transpose(out=labmat[:], in_=labf[:].to_broadcast([128, 128]), identity=bigt[:, 0:128])

    # ---- dense obj softplus (scalar; exp then ln(1+y) with accum) ----
    for s, (p_obj, t) in enumerate(objt):
        obje = pool.tile([p_obj, t.shape[1]], F32, tag=f"obje{s}")
        nc.scalar.activation(out=obje[:], in_=t[:], func=AF.Exp)
        objl = pool.tile([p_obj, t.shape[1]], F32, tag=f"objl{s}")
        nc.scalar.activation(
            out=objl[:], in_=obje[:], func=AF.Ln, bias=1.0,
            accum_out=stack[:p_obj, 6 * s + 4 : 6 * s + 5],
        )

    # ---- same-cell masks (vector): winners + min-label ----
    # lose_s[p,q] = (key_q == key_p) * utri[p,q]; win = 1 - max_q lose
    # minlab_p = min_q [ (key_q != key_p)*(BIG - lab_q) + lab_q ]
    minlab3 = pool.tile([NP, 3], F32, tag="minlab3")
    losev3 = pool.tile([NP, 3], F32, tag="losev3")
    for s in range(3):
        ne = pool.tile([128, 128], F32, tag=f"ne{s}")
        nc.vector.tensor_scalar(
            out=ne[:], in0=kmat[s][:], scalar1=keyf[:, s : s + 1], scalar2=None, op0=OP.not_equal
        )
        cnd = pool.tile([128, 128], F32, tag=f"cnd{s}")
        nc.vector.tensor_tensor(out=cnd[:], in0=ne[:], in1=blmat[:], op=OP.mult)
        nc.vector.tensor_tensor(out=cnd[:], in0=cnd[:], in1=labmat[:], op=OP.add)
        nc.vector.tensor_reduce(out=minlab3[:, s : s + 1], in_=cnd[:], axis=AX.X, op=OP.min)
        # lose = (1 - ne) * utri == eq * utri; fold (1-ne) via mult -1 add 1
        ls = pool.tile([128, 128], F32, tag=f"lose{s}")
        nc.vector.tensor_scalar(
            out=ls[:], in0=ne[:], scalar1=-1.0, scalar2=1.0, op0=OP.mult, op1=OP.add
        )
        nc.vector.tensor_tensor(out=ls[:], in0=ls[:], in1=utri, op=OP.mult)
        nc.vector.tensor_reduce(out=losev3[:, s : s + 1], in_=ls[:], axis=AX.X, op=OP.max)
    win3 = pool.tile([NP, 3], F32, tag="win3")
    nc.vector.tensor_scalar(out=win3[:], in0=losev3[:], scalar1=-1.0, scalar2=1.0, op0=OP.mult, op1=OP.add)
    nc.vector.tensor_copy(out=stv[:, :, 5], in_=win3[:])

    # ---- CE: lse at cells (scalar exp+accum, ln) + logit at min-label ----
    se3 = pool.tile([NP, 3], F32, tag="se3")
    rexp = pool.tile([NP, 3, C], F32, tag="rexp")
    for s in range(3):
        nc.scalar.activation(
            out=rexp[:, s, :], in_=rows[:, s, 5:], func=AF.Exp,
            accum_out=se3[:, s : s + 1],
        )
    nc.scalar.activation(out=stv[:, :, 0], in_=se3[:], func=AF.Ln)

    sel3 = pool.tile([NP, 3, C], F32, tag="sel3")
    nc.vector.tensor_tensor(
        out=sel3[:], in0=kct[:, None, 15:].to_broadcast([NP, 3, C]),
        in1=minlab3[:, :, None].to_broadcast([NP, 3, C]), op=OP.is_equal,
    )
    nc.vector.tensor_tensor(out=sel3[:], in0=sel3[:], in1=rows[:, :, 5:], op=OP.mult)
    nc.vector.tensor_reduce(out=stv[:, :, 1], in_=sel3[:], axis=AX.X, op=OP.add)

    # ---- smooth-L1 over gathered reg records (vector + scalar Abs) ----
    d12 = pool.tile([NP, 3, 4], F32, tag="d12")
    nc.vector.tensor_tensor(
        out=d12[:], in0=rows[:, :, 1:5], in1=btile[:, None, :].to_broadcast([NP, 3, 4]), op=OP.subtract
    )
    nc.scalar.activation(out=d12[:], in_=d12[:], func=AF.Abs)  # Abs is in set 6
    q12 = pool.tile([NP, 3, 4], F32, tag="q12")
    nc.vector.tensor_scalar_min(q12[:], d12[:], 1.0)
    h12 = pool.tile([NP, 3, 4], F32, tag="h12")
    nc.vector.tensor_scalar(out=h12[:], in0=q12[:], scalar1=-0.5, scalar2=None, op0=OP.mult)
    nc.vector.tensor_add(h12[:], h12[:], d12[:])
    nc.vector.tensor_mul(h12[:], h12[:], q12[:])
    sl13 = pool.tile([NP, 3], F32, tag="sl13")
    nc.vector.tensor_reduce(out=sl13[:], in_=h12[:], axis=AX.X, op=OP.add)
    nc.vector.tensor_scalar(out=stv[:, :, 2], in0=sl13[:], scalar1=0.25, scalar2=10.0, op0=OP.mult, op1=OP.min)
    # obj logit at cell
    nc.vector.tensor_copy(out=stv[:, :, 3], in_=rows[:, :, 0])

    # ---- mask positives, sum partitions via ones^T matmul, write out ----
    nc.vector.tensor_tensor(
        out=stv[:, :, 0:4], in0=stv[:, :, 0:4],
        in1=win3[:, :, None].to_broadcast([NP, 3, 4]), op=OP.mult,
    )
    fin_ps = fips.tile([1, NPART], F32, tag="fin_ps")
    nc.tensor.matmul(out=fin_ps[:], lhsT=onest[:], rhs=stack[:], start=True, stop=True)
    fin_sb = pool.tile([1, NPART], F32, tag="fin_sb")
    nc.vector.tensor_copy(out=fin_sb[:], in_=fin_ps[:])
    nc.sync.dma_start(out=out_ap.rearrange("(p f) -> p f", p=1), in_=fin_sb[:])

    for p in reversed(pools):
        p.release()


# ---------------------------------------------------------------------------
# host side
# ---------------------------------------------------------------------------

_CACHE = {}


def _build():
    if "nc" in _CACHE:
        return _CACHE["nc"]
    nc = bacc.Bacc(
        "TRN2",
        target_bir_lowering=False,
        debug=False,
        enable_asserts=False,
        num_devices=N_CORES,
    )
    ins = {}
    ins["rec"] = nc.dram_tensor("rec", (N_CELLS, REC_W), F32, kind="ExternalInput").ap()
    for s, (h, w) in enumerate(SCALES):
        ins[f"obj{s}"] = nc.dram_tensor(f"obj{s}", (B_SH * h * w,), F32, kind="ExternalInput").ap()
    ins["boxes"] = nc.dram_tensor("boxes", (B_SH, NBOX, 4), F32, kind="ExternalInput").ap()
    ins["labels"] = nc.dram_tensor("labels", (B_SH, NBOX), I32, kind="ExternalInput").ap()
    out = nc.dram_tensor("partials", (NPART,), F32, kind="ExternalOutput").ap()

    with tile.TileContext(nc) as tc:
        emit(tc, out, ins)
    nc.compile()
    _CACHE["nc"] = nc
    return nc


def make_records(inputs):
    """Full-batch per-cell records [B, sum(HW), 35]: obj, reg0..3, cls0..29."""
    per_scale = []
    for s, (h, w) in enumerate(SCALES):
        hw = h * w
        rec = np.empty((B_TOT, hw, REC_W), np.float32)
        rec[:, :, 0] = np.asarray(inputs[f"obj_p{s}"]).reshape(B_TOT, hw)
        rec[:, :, 1:5] = np.asarray(inputs[f"reg_p{s}"]).reshape(B_TOT, 4, hw).transpose(0, 2, 1)
        rec[:, :, 5:] = np.asarray(inputs[f"cls_p{s}"]).reshape(B_TOT, C, hw).transpose(0, 2, 1)
        per_scale.append(rec)
    return per_scale


def combine_partials(parts):
    """parts: [n_cores, 18] -> final [4] losses."""
    tot = np.asarray(parts, np.float64).sum(axis=0)
    cls_sum = reg_sum = obj_sum = 0.0
    for s, (h, w) in enumerate(SCALES):
        b = 6 * s
        lse, val, sl1, obj, sp, npos = tot[b : b + 6]
        npos = max(npos, 1.0)
        cls_sum += (lse - val) / npos * CLS_W
        reg_sum += sl1 / npos * REG_W
        obj_sum += (sp - obj) / (B_TOT * h * w) * OBJ_W
    cls_sum /= len(SCALES)
    reg_sum /= len(SCALES)
    obj_sum /= len(SCALES)
    total = cls_sum + reg_sum + obj_sum
    return np.array([total, cls_sum, reg_sum, obj_sum], np.float32)


TRACE = False
LAST_RESULT = None


def kernel(**inputs):
    global LAST_RESULT
    from concourse.bass_utils import run_bass_kernel_spmd

    nc = _build()
    per_scale = make_records(inputs)
    in_maps = []
    for c in range(N_CORES):
        lo, hi = c * B_SH, (c + 1) * B_SH
        m = {}
        m["rec"] = np.concatenate(
            [ps[lo:hi].reshape(-1, REC_W) for ps in per_scale], axis=0
        )
        for s, (h, w) in enumerate(SCALES):
            m[f"obj{s}"] = np.ascontiguousarray(
                np.asarray(inputs[f"obj_p{s}"][lo:hi]).reshape(-1)
            )
        m["boxes"] = np.ascontiguousarray(inputs["boxes"][lo:hi])
        m["labels"] = np.ascontiguousarray(inputs["labels"][lo:hi])
        in_maps.append(m)
    res = run_bass_kernel_spmd(
        nc, in_maps, core_ids=list(range(N_CORES)), trace=TRACE
    )
    LAST_RESULT = res
    parts = np.stack([np.asarray(r["partials"]) for r in res.results])
    return combine_partials(parts)


# revision 9
# speedup vs baseline: 1.8868x; 1.0697x over previous
"""DetectionLoss Trainium2 Bass kernel (v3 - sparse-only, fused masks).

Data-parallel over batch: 2 images per core x 8 cores; host sums per-box
partials (npos is a global normalizer, so per-core normalization is
impossible anyway - the sharding hint's "per-shard sums + counts").

The CE term only needs logsumexp at the <=128 positive cells, so there is
no dense cls work at all: the host packs per-cell records
[obj, reg0..3, cls0..29] (pure relayout) and the device gathers 35-wide
rows at the box cells. lse is exp+accum+ln on the gathered [128,3,30].
Dense work that remains: softplus over all obj logits (67KB).

Key tricks:
- one manual ACT table load of set 6 (natural_log_exp_and_others) serves
  every Exp and Ln (and Abs): auto-placement otherwise ping-pongs
  exp/ln tables at 1.28us per load.
- single-reduce winner+minlab: minv_p = min_q [ ne*BIG + lab_q - 64*utri ].
  Same-cell later boxes contribute lab-64 < 0, so win = (minv >= 0) and
  minv == min-label exactly when p is a winner. One [128,128] compare and
  one reduce per scale instead of two of each.
- boxes/labels/consts packed into one [128,50] DMA; labels pre-cast to f32.
- output is the raw [128,18] per-box partial matrix; host masks nothing
  (win already multiplied on device) and just sums.
"""

import numpy as np

import concourse.bass as bass
import concourse.tile as tile
from concourse import bacc, mybir

F32 = mybir.dt.float32
I32 = mybir.dt.int32
AF = mybir.ActivationFunctionType
OP = mybir.AluOpType
AX = mybir.AxisListType

B_TOT = 16
N_CORES = 8
B_SH = B_TOT // N_CORES
NBOX = 64
NP = B_SH * NBOX  # 128 partitions: (image, box)
C = 30
SCALES = [(80, 80), (40, 40), (20, 20)]
BIG = 1.0e9
REC_W = 5 + C  # obj, reg0..3, cls0..29
N_CELLS = sum(B_SH * h * w for h, w in SCALES)  # 16800
REC_BASE = [0, B_SH * 6400, B_SH * 6400 + B_SH * 1600]

CLS_W, REG_W, OBJ_W = 1.0, 5.0, 1.0
NPART = 18  # per scale s, cols 6s + [lse, clsval, sl1, obj, softplus, npos]

# pack layout: [kc (45) | boxes (4) | labels-as-f32 (1)] = 50 cols
KC_W = 15 + C
PK_W = KC_W + 5

# act_info.json set 6 = natural_log_exp_and_others: serves Exp, Ln, Abs
ACT_SET_EXP_LN = 6


def _kc_const():
    p = np.arange(128)
    bvec = (p >= NBOX).astype(np.float32)
    kc = np.zeros((128, KC_W), np.float32)
    for s, (h, w) in enumerate(SCALES):
        hw = h * w
        kc[:, 0 + s] = w          # x multiplier
        kc[:, 3 + s] = h          # y multiplier
        kc[:, 6 + s] = w - 1      # x clip
        kc[:, 9 + s] = h - 1      # y clip
        kc[:, 12 + s] = bvec * hw + REC_BASE[s]  # record-row offset
    kc[:, 15:] = np.arange(C, dtype=np.float32)[None, :]  # iota over classes
    return kc


def _big_const():
    ident = np.eye(128, dtype=np.float32)
    m64utri = -64.0 * np.triu(np.ones((128, 128), np.float32), 1)
    return np.concatenate([ident, m64utri], axis=1)  # [128, 256]


def emit(tc: tile.TileContext, outs, ins):
    """outs: partials AP [128,18]; ins: dict name -> AP (per-core shapes)."""
    nc = tc.nc

    big_h = nc.inline_tensor(_big_const(), name="cbig")

    pools = []

    def mkpool(**kw):
        p = tc.alloc_tile_pool(**kw)
        pools.append(p)
        return p

    pool = mkpool(name="sb", bufs=1)
    tps = mkpool(name="tps", bufs=1, space="PSUM")

    # ---- single activation-table load serving all Exp AND Ln ops ----
    nc.scalar.add_instruction(mybir.InstLoadActFuncSet(
        name=nc.scalar.bass.get_next_instruction_name(),
        act_func_set_id=ACT_SET_EXP_LN,
        engine=mybir.EngineType.Activation, ins=[], outs=[]))

    # ---- input loads ----
    pk = pool.tile([128, PK_W], F32, tag="pk")
    nc.sync.dma_start(out=pk[:], in_=ins["pack"])
    kct = pk[:, 0:KC_W]
    btile = pk[:, KC_W : KC_W + 4]
    labf = pk[:, KC_W + 4 : KC_W + 5]
    # gpsimd queue (idle until the gathers): transpose identity + -64*utri
    bigt = pool.tile([128, 256], F32, tag="bigt")
    nc.gpsimd.dma_start(out=bigt[:], in_=big_h.ap())
    m64utri = bigt[:, 128:256]
    # scalar queue: dense obj logits
    objt = []
    for s, (h, w) in enumerate(SCALES):
        n = B_SH * h * w
        p_obj = 128 if s < 2 else 32
        t = pool.tile([p_obj, n // p_obj], F32, tag=f"objt{s}")
        nc.scalar.dma_start(out=t[:], in_=ins[f"obj{s}"].rearrange("(p f) -> p f", p=p_obj))
        objt.append((p_obj, t))

    stack = pool.tile([128, NPART], F32, tag="stack")
    nc.vector.memset(stack[:], 0.0)
    stv = stack[:].rearrange("p (s j) -> p s j", j=6)

    # ---- box -> cell/key indices (vector) ----
    # floor(x) = round-to-nearest(x - 0.5): HW f32->i32 convert rounds.
    kxy = kct[:, 0:6].rearrange("p (c s) -> p c s", c=2)
    kxy_clip = kct[:, 6:12].rearrange("p (c s) -> p c s", c=2)
    gr = pool.tile([NP, 2, 3], F32, tag="gr")
    nc.vector.tensor_tensor(
        out=gr[:], in0=btile[:, 0:2, None].to_broadcast([NP, 2, 3]), in1=kxy, op=OP.mult
    )
    nc.vector.tensor_scalar(out=gr[:], in0=gr[:], scalar1=-0.5, scalar2=None, op0=OP.add)
    gi = pool.tile([NP, 2, 3], I32, tag="gi")
    nc.vector.tensor_copy(out=gi[:], in_=gr[:])
    gf = pool.tile([NP, 2, 3], F32, tag="gf")
    nc.vector.tensor_copy(out=gf[:], in_=gi[:])
    nc.vector.tensor_tensor(out=gf[:], in0=gf[:], in1=kxy_clip, op=OP.min)

    keyf = pool.tile([NP, 3], F32, tag="keyf")
    nc.vector.tensor_tensor(out=keyf[:], in0=gf[:, 1, :], in1=kct[:, 0:3], op=OP.mult)
    nc.vector.tensor_add(keyf[:], keyf[:], gf[:, 0, :])
    nc.vector.tensor_add(keyf[:], keyf[:], kct[:, 12:15])
    keyi = pool.tile([NP, 3], I32, tag="keyi")
    nc.vector.tensor_copy(out=keyi[:], in_=keyf[:])

    # ---- record gathers (gpsimd): one [128, 35] row per box per scale ----
    rows = pool.tile([NP, 3, REC_W], F32, tag="rows")
    for s in range(3):
        nc.gpsimd.indirect_dma_start(
            out=rows[:, s, :],
            out_offset=None,
            in_=ins["rec"],
            in_offset=bass.IndirectOffsetOnAxis(ap=keyi[:, s : s + 1], axis=0),
        )

    # ---- PE broadcast-transposes: labmat first, then per-scale keys ----
    labmat = tps.tile([128, 128], F32, tag="labmat")
    nc.tensor.transpose(out=labmat[:], in_=labf.to_broadcast([128, 128]), identity=bigt[:, 0:128])
    kmat = []
    for s in range(3):
        km = tps.tile([128, 128], F32, tag=f"kmat{s}")
        nc.tensor.transpose(
            out=km[:], in_=keyf[:, s : s + 1].to_broadcast([128, 128]), identity=bigt[:, 0:128]
        )
        kmat.append(km)

    # ---- dense obj softplus (scalar; exp then ln(1+y) with accum) ----
    for s, (p_obj, t) in enumerate(objt):
        obje = pool.tile([p_obj, t.shape[1]], F32, tag=f"obje{s}")
        nc.scalar.activation(out=obje[:], in_=t[:], func=AF.Exp)
        objl = pool.tile([p_obj, t.shape[1]], F32, tag=f"objl{s}")
        nc.scalar.activation(
            out=objl[:], in_=obje[:], func=AF.Ln, bias=1.0,
            accum_out=stack[:p_obj, 6 * s + 4 : 6 * s + 5],
        )

    # ---- same-cell masks (vector): single reduce gives winner AND minlab ----
    # lu[p,q] = lab_q - 64*utri[p,q]
    # minv_p  = min_q [ (key_q != key_p)*BIG + lu[p,q] ]
    #   winner (no later same-cell box): minv = min-label in [0, 30)
    #   loser: minv = lab_j - 64 in [-64, -35)  -> win = (minv >= 0)
    lu = pool.tile([128, 128], F32, tag="lu")
    nc.vector.tensor_tensor(out=lu[:], in0=labmat[:], in1=m64utri, op=OP.add)
    minv3 = pool.tile([NP, 3], F32, tag="minv3")
    for s in range(3):
        ne = pool.tile([128, 128], F32, tag=f"ne{s}")
        nc.vector.tensor_scalar(
            out=ne[:], in0=kmat[s][:], scalar1=keyf[:, s : s + 1], scalar2=None, op0=OP.not_equal
        )
        nc.vector.tensor_scalar(out=ne[:], in0=ne[:], scalar1=BIG, scalar2=None, op0=OP.mult)
        nc.vector.tensor_tensor(out=ne[:], in0=ne[:], in1=lu[:], op=OP.add)
        nc.vector.tensor_reduce(out=minv3[:, s : s + 1], in_=ne[:], axis=AX.X, op=OP.min)
    win3 = pool.tile([NP, 3], F32, tag="win3")
    nc.vector.tensor_scalar(out=win3[:], in0=minv3[:], scalar1=0.0, scalar2=None, op0=OP.is_ge)
    nc.vector.tensor_copy(out=stv[:, :, 5], in_=win3[:])

    # ---- CE: lse at cells (scalar exp+accum, ln) + logit at min-label ----
    se3 = pool.tile([NP, 3], F32, tag="se3")
    rexp = pool.tile([NP, 3, C], F32, tag="rexp")
    for s in range(3):
        nc.scalar.activation(
            out=rexp[:, s, :], in_=rows[:, s, 5:], func=AF.Exp,
            accum_out=se3[:, s : s + 1],
        )
    nc.scalar.activation(out=stv[:, :, 0], in_=se3[:], func=AF.Ln)

    # ---- smooth-L1 over gathered reg records (vector; |d| via max(d,-d)) ----
    d12 = pool.tile([NP, 3, 4], F32, tag="d12")
    nc.vector.tensor_tensor(
        out=d12[:], in0=rows[:, :, 1:5], in1=btile[:, None, :].to_broadcast([NP, 3, 4]), op=OP.subtract
    )
    dn12 = pool.tile([NP, 3, 4], F32, tag="dn12")
    nc.vector.tensor_tensor(
        out=dn12[:], in0=btile[:, None, :].to_broadcast([NP, 3, 4]), in1=rows[:, :, 1:5], op=OP.subtract
    )
    nc.vector.tensor_tensor(out=d12[:], in0=d12[:], in1=dn12[:], op=OP.max)
    q12 = pool.tile([NP, 3, 4], F32, tag="q12")
    nc.vector.tensor_scalar_min(q12[:], d12[:], 1.0)
    h12 = pool.tile([NP, 3, 4], F32, tag="h12")
    nc.vector.tensor_scalar(out=h12[:], in0=q12[:], scalar1=-0.5, scalar2=None, op0=OP.mult)
    nc.vector.tensor_add(h12[:], h12[:], d12[:])
    nc.vector.tensor_mul(h12[:], h12[:], q12[:])
    sl13 = pool.tile([NP, 3], F32, tag="sl13")
    nc.vector.tensor_reduce(out=sl13[:], in_=h12[:], axis=AX.X, op=OP.add)
    nc.vector.tensor_scalar(out=stv[:, :, 2], in0=sl13[:], scalar1=0.25, scalar2=10.0, op0=OP.mult, op1=OP.min)
    # obj logit at cell
    nc.vector.tensor_copy(out=stv[:, :, 3], in_=rows[:, :, 0])

    # ---- cls logit at min-label (0 for losers: minv < 0 never matches iota) ----
    sel3 = pool.tile([NP, 3, C], F32, tag="sel3")
    nc.vector.tensor_tensor(
        out=sel3[:], in0=kct[:, None, 15:].to_broadcast([NP, 3, C]),
        in1=minv3[:, :, None].to_broadcast([NP, 3, C]), op=OP.is_equal,
    )
    nc.vector.tensor_tensor(out=sel3[:], in0=sel3[:], in1=rows[:, :, 5:], op=OP.mult)
    nc.vector.tensor_reduce(out=stv[:, :, 1], in_=sel3[:], axis=AX.X, op=OP.add)

    # ---- mask positives; ship the [128,18] partial matrix, host sums ----
    nc.vector.tensor_tensor(
        out=stv[:, :, 0:4], in0=stv[:, :, 0:4],
        in1=win3[:, :, None].to_broadcast([NP, 3, 4]), op=OP.mult,
    )
    nc.sync.dma_start(out=outs, in_=stack[:])

    for p in reversed(pools):
        p.release()


# ---------------------------------------------------------------------------
# host side
# ---------------------------------------------------------------------------

_CACHE = {}


def _build():
    if "nc" in _CACHE:
        return _CACHE["nc"]
    nc = bacc.Bacc(
        "TRN2",
        target_bir_lowering=False,
        debug=False,
        enable_asserts=False,
        num_devices=N_CORES,
    )
    ins = {}
    ins["rec"] = nc.dram_tensor("rec", (N_CELLS, REC_W), F32, kind="ExternalInput").ap()
    for s, (h, w) in enumerate(SCALES):
        ins[f"obj{s}"] = nc.dram_tensor(f"obj{s}", (B_SH * h * w,), F32, kind="ExternalInput").ap()
    ins["pack"] = nc.dram_tensor("pack", (128, PK_W), F32, kind="ExternalInput").ap()
    out = nc.dram_tensor("partials", (128, NPART), F32, kind="ExternalOutput").ap()

    with tile.TileContext(nc) as tc:
        emit(tc, out, ins)
    nc.compile()
    _CACHE["nc"] = nc
    return nc


def make_records(inputs):
    """Full-batch per-cell records [B, HW_s, 35]: obj, reg0..3, cls0..29."""
    per_scale = []
    for s, (h, w) in enumerate(SCALES):
        hw = h * w
        rec = np.empty((B_TOT, hw, REC_W), np.float32)
        rec[:, :, 0] = np.asarray(inputs[f"obj_p{s}"]).reshape(B_TOT, hw)
        rec[:, :, 1:5] = np.asarray(inputs[f"reg_p{s}"]).reshape(B_TOT, 4, hw).transpose(0, 2, 1)
        rec[:, :, 5:] = np.asarray(inputs[f"cls_p{s}"]).reshape(B_TOT, C, hw).transpose(0, 2, 1)
        per_scale.append(rec)
    return per_scale


def combine_partials(parts):
    """parts: [n_cores, 128, 18] -> final [4] losses."""
    tot = np.asarray(parts, np.float64).sum(axis=(0, 1))
    cls_sum = reg_sum = obj_sum = 0.0
    for s, (h, w) in enumerate(SCALES):
        b = 6 * s
        lse, val, sl1, obj, sp, npos = tot[b : b + 6]
        npos = max(npos, 1.0)
        cls_sum += (lse - val) / npos * CLS_W
        reg_sum += sl1 / npos * REG_W
        obj_sum += (sp - obj) / (B_TOT * h * w) * OBJ_W
    cls_sum /= len(SCALES)
    reg_sum /= len(SCALES)
    obj_sum /= len(SCALES)
    total = cls_sum + reg_sum + obj_sum
    return np.array([total, cls_sum, reg_sum, obj_sum], np.float32)


TRACE = False
LAST_RESULT = None

_KC = _kc_const()


def kernel(**inputs):
    global LAST_RESULT
    from concourse.bass_utils import run_bass_kernel_spmd

    nc = _build()
    per_scale = make_records(inputs)
    boxes = np.asarray(inputs["boxes"], np.float32)
    labels = np.asarray(inputs["labels"])
    in_maps = []
    for c in range(N_CORES):
        lo, hi = c * B_SH, (c + 1) * B_SH
        m = {}
        m["rec"] = np.concatenate(
            [ps[lo:hi].reshape(-1, REC_W) for ps in per_scale], axis=0
        )
        for s in range(3):
            m[f"obj{s}"] = np.ascontiguousarray(
                np.asarray(inputs[f"obj_p{s}"][lo:hi]).reshape(-1)
            )
        pack = np.empty((128, PK_W), np.float32)
        pack[:, :KC_W] = _KC
        pack[:, KC_W : KC_W + 4] = boxes[lo:hi].reshape(NP, 4)
        pack[:, KC_W + 4] = labels[lo:hi].reshape(NP).astype(np.float32)
        m["pack"] = pack
        in_maps.append(m)
    res = run_bass_kernel_spmd(
        nc, in_maps, core_ids=list(range(N_CORES)), trace=TRACE
    )
    LAST_RESULT = res
    parts = np.stack([np.asarray(r["partials"]) for r in res.results])
    return combine_partials(parts)


# revision 17
# speedup vs baseline: 1.9878x; 1.0535x over previous
"""DetectionLoss Trainium2 Bass kernel (v3 - sparse-only, fused masks).

Data-parallel over batch: 2 images per core x 8 cores; host sums per-box
partials (npos is a global normalizer, so per-core normalization is
impossible anyway - the sharding hint's "per-shard sums + counts").

The CE term only needs logsumexp at the <=128 positive cells, so there is
no dense cls work at all: the host packs per-cell records
[obj, reg0..3, cls0..29] (pure relayout) and the device gathers 35-wide
rows at the box cells. lse is exp+accum+ln on the gathered [128,3,30].
Dense work that remains: softplus over all obj logits (67KB).

Key tricks:
- one manual ACT table load of set 6 (natural_log_exp_and_others) serves
  every Exp and Ln (and Abs): auto-placement otherwise ping-pongs
  exp/ln tables at 1.28us per load.
- single-reduce winner+minlab: minv_p = min_q [ ne*BIG + lab_q - 64*utri ].
  Same-cell later boxes contribute lab-64 < 0, so win = (minv >= 0) and
  minv == min-label exactly when p is a winner. One [128,128] compare and
  one reduce per scale instead of two of each.
- boxes/labels/consts packed into one [128,50] DMA; labels pre-cast to f32.
- output is the raw [128,18] per-box partial matrix; host masks nothing
  (win already multiplied on device) and just sums.
"""

import numpy as np

import concourse.bass as bass
import concourse.tile as tile
from concourse import bacc, mybir
from concourse.tile_rust import add_dep_helper

F32 = mybir.dt.float32
I32 = mybir.dt.int32
AF = mybir.ActivationFunctionType
OP = mybir.AluOpType
AX = mybir.AxisListType

B_TOT = 16
N_CORES = 8
B_SH = B_TOT // N_CORES
NBOX = 64
NP = B_SH * NBOX  # 128 partitions: (image, box)
C = 30
SCALES = [(80, 80), (40, 40), (20, 20)]
BIG = 1.0e9
REC_W = 5 + C  # obj, reg0..3, cls0..29
N_CELLS = sum(B_SH * h * w for h, w in SCALES)  # 16800
REC_BASE = [0, B_SH * 6400, B_SH * 6400 + B_SH * 1600]

CLS_W, REG_W, OBJ_W = 1.0, 5.0, 1.0
NPART = 18  # per scale s, cols 6s + [lse, clsval, sl1, obj, softplus, npos]

# pack layout: [kc (45) | boxes (4) | labels-as-f32 (1)] = 50 cols
KC_W = 15 + C
PK_W = KC_W + 5

# act_info.json set 6 = natural_log_exp_and_others: serves Exp, Ln, Abs
ACT_SET_EXP_LN = 6


def _kc_const():
    p = np.arange(128)
    bvec = (p >= NBOX).astype(np.float32)
    kc = np.zeros((128, KC_W), np.float32)
    for s, (h, w) in enumerate(SCALES):
        hw = h * w
        kc[:, 0 + s] = w          # x multiplier
        kc[:, 3 + s] = h          # y multiplier
        kc[:, 6 + s] = w - 1      # x clip
        kc[:, 9 + s] = h - 1      # y clip
        kc[:, 12 + s] = bvec * hw + REC_BASE[s]  # record-row offset
    kc[:, 15:] = np.arange(C, dtype=np.float32)[None, :]  # iota over classes
    return kc


def _big_const():
    ident = np.eye(128, dtype=np.float32)
    m64utri = -64.0 * np.triu(np.ones((128, 128), np.float32), 1)
    return np.concatenate([ident, m64utri], axis=1)  # [128, 256]


def emit(tc: tile.TileContext, outs, ins):
    """outs: partials AP [128,18]; ins: dict name -> AP (per-core shapes)."""
    nc = tc.nc

    big_h = nc.inline_tensor(_big_const(), name="cbig")

    pools = []

    def mkpool(**kw):
        p = tc.alloc_tile_pool(**kw)
        pools.append(p)
        return p

    pool = mkpool(name="sb", bufs=1)
    tps = mkpool(name="tps", bufs=1, space="PSUM")

    # ---- single activation-table load serving all Exp AND Ln ops ----
    nc.scalar.add_instruction(mybir.InstLoadActFuncSet(
        name=nc.scalar.bass.get_next_instruction_name(),
        act_func_set_id=ACT_SET_EXP_LN,
        engine=mybir.EngineType.Activation, ins=[], outs=[]))

    # ---- input loads ----
    pk = pool.tile([128, PK_W], F32, tag="pk")
    nc.sync.dma_start(out=pk[:], in_=ins["pack"])
    kct = pk[:, 0:KC_W]
    btile = pk[:, KC_W : KC_W + 4]
    labf = pk[:, KC_W + 4 : KC_W + 5]
    # gpsimd queue (idle until the gathers): transpose identity + -64*utri
    bigt = pool.tile([128, 256], F32, tag="bigt")
    nc.gpsimd.dma_start(out=bigt[:], in_=big_h.ap())
    m64utri = bigt[:, 128:256]
    # scalar queue: dense obj logits
    objt = []
    for s, (h, w) in enumerate(SCALES):
        n = B_SH * h * w
        p_obj = 128 if s < 2 else 32
        t = pool.tile([p_obj, n // p_obj], F32, tag=f"objt{s}")
        nc.scalar.dma_start(out=t[:], in_=ins[f"obj{s}"].rearrange("(p f) -> p f", p=p_obj))
        objt.append((p_obj, t))

    stack = pool.tile([128, NPART], F32, tag="stack")
    nc.vector.memset(stack[:], 0.0)
    stv = stack[:].rearrange("p (s j) -> p s j", j=6)

    # ---- box -> cell/key indices (vector) ----
    # floor(x) = round-to-nearest(x - 0.5): HW f32->i32 convert rounds.
    kxy = kct[:, 0:6].rearrange("p (c s) -> p c s", c=2)
    kxy_clip = kct[:, 6:12].rearrange("p (c s) -> p c s", c=2)
    gr = pool.tile([NP, 2, 3], F32, tag="gr")
    nc.vector.tensor_tensor(
        out=gr[:], in0=btile[:, 0:2, None].to_broadcast([NP, 2, 3]), in1=kxy, op=OP.mult
    )
    nc.vector.tensor_scalar(out=gr[:], in0=gr[:], scalar1=-0.5, scalar2=None, op0=OP.add)
    gi = pool.tile([NP, 2, 3], I32, tag="gi")
    nc.vector.tensor_copy(out=gi[:], in_=gr[:])
    gf = pool.tile([NP, 2, 3], F32, tag="gf")
    nc.vector.tensor_copy(out=gf[:], in_=gi[:])
    nc.vector.tensor_tensor(out=gf[:], in0=gf[:], in1=kxy_clip, op=OP.min)

    keyf = pool.tile([NP, 3], F32, tag="keyf")
    nc.vector.tensor_tensor(out=keyf[:], in0=gf[:, 1, :], in1=kct[:, 0:3], op=OP.mult)
    nc.vector.tensor_add(keyf[:], keyf[:], gf[:, 0, :])
    nc.vector.tensor_add(keyf[:], keyf[:], kct[:, 12:15])
    keyi = pool.tile([NP, 3], I32, tag="keyi")
    keyi_i = nc.vector.tensor_copy(out=keyi[:], in_=keyf[:])

    # ---- record gathers (gpsimd): indirect DMA honors ONE offset per
    # partition, so one [128, 35] gather per scale ----
    rows = pool.tile([NP, 3, REC_W], F32, tag="rows")
    for s in range(3):
        nc.gpsimd.indirect_dma_start(
            out=rows[:, s, :],
            out_offset=None,
            in_=ins["rec"],
            in_offset=bass.IndirectOffsetOnAxis(ap=keyi[:, s : s + 1], axis=0),
        )

    # ---- PE broadcast-transposes: labmat first, then per-scale keys ----
    labmat = tps.tile([128, 128], F32, tag="labmat")
    nc.tensor.transpose(out=labmat[:], in_=labf.to_broadcast([128, 128]), identity=bigt[:, 0:128])
    kmat = []
    for s in range(3):
        km = tps.tile([128, 128], F32, tag=f"kmat{s}")
        nc.tensor.transpose(
            out=km[:], in_=keyf[:, s : s + 1].to_broadcast([128, 128]), identity=bigt[:, 0:128]
        )
        kmat.append(km)

    # ---- dense obj softplus (scalar; exp then ln(1+y) with accum) ----
    for s, (p_obj, t) in enumerate(objt):
        obje = pool.tile([p_obj, t.shape[1]], F32, tag=f"obje{s}")
        nc.scalar.activation(out=obje[:], in_=t[:], func=AF.Exp)
        objl = pool.tile([p_obj, t.shape[1]], F32, tag=f"objl{s}")
        nc.scalar.activation(
            out=objl[:], in_=obje[:], func=AF.Ln, bias=1.0,
            accum_out=stack[:p_obj, 6 * s + 4 : 6 * s + 5],
        )

    # ---- same-cell masks (vector): single reduce gives winner AND minlab ----
    # lu[p,q] = lab_q - 64*utri[p,q]
    # minv_p  = min_q [ (key_q != key_p)*BIG + lu[p,q] ]
    #   winner (no later same-cell box): minv = min-label in [0, 30)
    #   loser: minv = lab_j - 64 in [-64, -35)  -> win = (minv >= 0)
    lu = pool.tile([128, 128], F32, tag="lu")
    lu_i = nc.vector.tensor_tensor(out=lu[:], in0=labmat[:], in1=m64utri, op=OP.add)
    # keep the scheduler from hoisting lu (waits on the labmat matmul) into
    # the middle of the box chain - it head-of-line blocks keyi otherwise
    add_dep_helper(lu_i.ins, keyi_i.ins, reason="order: box chain first")
    minv3 = pool.tile([NP, 3], F32, tag="minv3")
    for s in range(3):
        ne = pool.tile([128, 128], F32, tag=f"ne{s}")
        nc.vector.tensor_scalar(
            out=ne[:], in0=kmat[s][:], scalar1=keyf[:, s : s + 1], scalar2=BIG,
            op0=OP.not_equal, op1=OP.mult,
        )
        nc.vector.tensor_tensor(out=ne[:], in0=ne[:], in1=lu[:], op=OP.add)
        nc.vector.tensor_reduce(out=minv3[:, s : s + 1], in_=ne[:], axis=AX.X, op=OP.min)
    win3 = pool.tile([NP, 3], F32, tag="win3")
    nc.vector.tensor_scalar(out=win3[:], in0=minv3[:], scalar1=0.0, scalar2=None, op0=OP.is_ge)
    nc.vector.tensor_copy(out=stv[:, :, 5], in_=win3[:])

    # ---- CE: lse at cells (scalar exp+accum, ln) + logit at min-label ----
    se3 = pool.tile([NP, 3], F32, tag="se3")
    rexp = pool.tile([NP, 3, C], F32, tag="rexp")
    for s in range(3):
        nc.scalar.activation(
            out=rexp[:, s, :], in_=rows[:, s, 5:], func=AF.Exp,
            accum_out=se3[:, s : s + 1],
        )
    nc.scalar.activation(out=stv[:, :, 0], in_=se3[:], func=AF.Ln)

    # ---- smooth-L1 over gathered reg records (vector; |d| via max(d,-d)) ----
    d12 = pool.tile([NP, 3, 4], F32, tag="d12")
    nc.vector.tensor_tensor(
        out=d12[:], in0=rows[:, :, 1:5], in1=btile[:, None, :].to_broadcast([NP, 3, 4]), op=OP.subtract
    )
    dn12 = pool.tile([NP, 3, 4], F32, tag="dn12")
    nc.vector.tensor_tensor(
        out=dn12[:], in0=btile[:, None, :].to_broadcast([NP, 3, 4]), in1=rows[:, :, 1:5], op=OP.subtract
    )
    nc.vector.tensor_tensor(out=d12[:], in0=d12[:], in1=dn12[:], op=OP.max)
    q12 = pool.tile([NP, 3, 4], F32, tag="q12")
    nc.vector.tensor_scalar_min(q12[:], d12[:], 1.0)
    h12 = pool.tile([NP, 3, 4], F32, tag="h12")
    nc.vector.tensor_scalar(out=h12[:], in0=q12[:], scalar1=-0.5, scalar2=None, op0=OP.mult)
    nc.vector.tensor_add(h12[:], h12[:], d12[:])
    nc.vector.tensor_mul(h12[:], h12[:], q12[:])
    sl13 = pool.tile([NP, 3], F32, tag="sl13")
    nc.vector.tensor_reduce(out=sl13[:], in_=h12[:], axis=AX.X, op=OP.add)
    nc.vector.tensor_scalar(out=stv[:, :, 2], in0=sl13[:], scalar1=0.25, scalar2=10.0, op0=OP.mult, op1=OP.min)
    # obj logit at cell
    nc.vector.tensor_copy(out=stv[:, :, 3], in_=rows[:, :, 0])

    # ---- cls logit at min-label (0 for losers: minv < 0 never matches iota) ----
    sel3 = pool.tile([NP, 3, C], F32, tag="sel3")
    nc.vector.tensor_tensor(
        out=sel3[:], in0=kct[:, None, 15:].to_broadcast([NP, 3, C]),
        in1=minv3[:, :, None].to_broadcast([NP, 3, C]), op=OP.is_equal,
    )
    nc.vector.tensor_tensor(out=sel3[:], in0=sel3[:], in1=rows[:, :, 5:], op=OP.mult)
    nc.vector.tensor_reduce(out=stv[:, :, 1], in_=sel3[:], axis=AX.X, op=OP.add)

    # ---- ship the raw [128,18] per-box partials; host does the
    # win-weighted sum (the hint's "per-shard sums + counts") ----
    nc.sync.dma_start(out=outs, in_=stack[:])

    for p in reversed(pools):
        p.release()


# ---------------------------------------------------------------------------
# host side
# ---------------------------------------------------------------------------

_CACHE = {}


def _build():
    if "nc" in _CACHE:
        return _CACHE["nc"]
    nc = bacc.Bacc(
        "TRN2",
        target_bir_lowering=False,
        debug=False,
        enable_asserts=False,
        num_devices=N_CORES,
    )
    ins = {}
    ins["rec"] = nc.dram_tensor("rec", (N_CELLS, REC_W), F32, kind="ExternalInput").ap()
    for s, (h, w) in enumerate(SCALES):
        ins[f"obj{s}"] = nc.dram_tensor(f"obj{s}", (B_SH * h * w,), F32, kind="ExternalInput").ap()
    ins["pack"] = nc.dram_tensor("pack", (128, PK_W), F32, kind="ExternalInput").ap()
    out = nc.dram_tensor("partials", (128, NPART), F32, kind="ExternalOutput").ap()

    with tile.TileContext(nc) as tc:
        emit(tc, out, ins)
    nc.compile()
    _CACHE["nc"] = nc
    return nc


def make_records(inputs):
    """Full-batch per-cell records [B, HW_s, 35]: obj, reg0..3, cls0..29."""
    per_scale = []
    for s, (h, w) in enumerate(SCALES):
        hw = h * w
        rec = np.empty((B_TOT, hw, REC_W), np.float32)
        rec[:, :, 0] = np.asarray(inputs[f"obj_p{s}"]).reshape(B_TOT, hw)
        rec[:, :, 1:5] = np.asarray(inputs[f"reg_p{s}"]).reshape(B_TOT, 4, hw).transpose(0, 2, 1)
        rec[:, :, 5:] = np.asarray(inputs[f"cls_p{s}"]).reshape(B_TOT, C, hw).transpose(0, 2, 1)
        per_scale.append(rec)
    return per_scale


def combine_partials(parts):
    """parts: [n_cores, 128, 18] raw per-box partials -> final [4] losses.
    Device ships unmasked values; the win flag (col 6s+5) weights them here."""
    p = np.asarray(parts, np.float64).reshape(-1, 3, 6)
    win = p[:, :, 5:6]
    tot = np.concatenate([(p[:, :, 0:4] * win), p[:, :, 4:6]], axis=2).sum(axis=0)
    cls_sum = reg_sum = obj_sum = 0.0
    for s, (h, w) in enumerate(SCALES):
        lse, val, sl1, obj, sp, npos = tot[s]
        npos = max(npos, 1.0)
        cls_sum += (lse - val) / npos * CLS_W
        reg_sum += sl1 / npos * REG_W
        obj_sum += (sp - obj) / (B_TOT * h * w) * OBJ_W
    cls_sum /= len(SCALES)
    reg_sum /= len(SCALES)
    obj_sum /= len(SCALES)
    total = cls_sum + reg_sum + obj_sum
    return np.array([total, cls_sum, reg_sum, obj_sum], np.float32)


TRACE = False
LAST_RESULT = None

_KC = _kc_const()


def kernel(**inputs):
    global LAST_RESULT
    from concourse.bass_utils import run_bass_kernel_spmd

    nc = _build()
    per_scale = make_records(inputs)
    boxes = np.asarray(inputs["boxes"], np.float32)
    labels = np.asarray(inputs["labels"])
    in_maps = []
    for c in range(N_CORES):
        lo, hi = c * B_SH, (c + 1) * B_SH
        m = {}
        m["rec"] = np.concatenate(
            [ps[lo:hi].reshape(-1, REC_W) for ps in per_scale], axis=0
        )
        for s in range(3):
            m[f"obj{s}"] = np.ascontiguousarray(
                np.asarray(inputs[f"obj_p{s}"][lo:hi]).reshape(-1)
            )
        pack = np.empty((128, PK_W), np.float32)
        pack[:, :KC_W] = _KC
        pack[:, KC_W : KC_W + 4] = boxes[lo:hi].reshape(NP, 4)
        pack[:, KC_W + 4] = labels[lo:hi].reshape(NP).astype(np.float32)
        m["pack"] = pack
        in_maps.append(m)
    res = run_bass_kernel_spmd(
        nc, in_maps, core_ids=list(range(N_CORES)), trace=TRACE
    )
    LAST_RESULT = res
    parts = np.stack([np.asarray(r["partials"]) for r in res.results])
    return combine_partials(parts)
